# revision 11
# baseline (speedup 1.0000x reference)
"""Trainium2 Bass kernel for a 2-layer dual-branch GCN (nn_ATACGCN).

reference:
    zs, zu, za = split(z)
    ys = elu(adj @ (elu(zs) @ W0) + b0); ys = elu(adj @ (ys @ W1) + b1)
    yu = elu(adj @ (elu(zu) @ W0) + b0); yu = elu(adj @ (yu @ W1) + b1)
    out = concat(ys, yu, za) @ Wl + bl

Strategy: 1D row-shard of the node dimension across 8 NeuronCores. Both
branches share weights, so they are fused into one 128-wide feature block
(block-diagonal W). Each core streams its [16384, 2048] slab of adj^T (bf16)
from HBM twice (once per GCN layer) and accumulates Y^T = H^T @ adjT in
PSUM. The only cross-core exchange is a 1 MiB/rank AllGather of the layer-1
input features H1 between the two layers.

elu(x) is composed as min(exp(x) - 1, max(x, 0)).
"""

import numpy as np
import ml_dtypes

BF16 = ml_dtypes.bfloat16

# Problem constants (hardcoded per harness contract).
N = 16384      # nodes
D = 64         # per-branch width
OUT = 64       # output width
L = 2          # gcn layers
N_CORES = 8
P = 128        # SBUF partitions
RPC = N // N_CORES          # rows (nodes) per core
KT = N // P                 # contraction tiles
T_PC = RPC // P             # node tiles per core
CW_A = min(1024, N)         # stage-A chunk width (input elu)
CW_Y = min(512, RPC)        # adj-matmul PSUM chunk width
NCH_Y = RPC // CW_Y         # PSUM chunks per core


def build_kernel_body(tc, ins, outs, n_cores=N_CORES, n=N, with_collective=True,
                      adj_dtype="bf16"):
    """Emit the per-core Tile program.

    ins/outs: dicts name -> bass.AP of the DRAM I/O tensors:
      adjt [n, rpc] bf16/fp8, zsut [128, n] f32, zat [64, rpc] bf16,
      wbd [128, 2*128] bf16, wlsu [128, 64] bf16, wlza [64, 64] bf16,
      blr [1, 64] bf16, bias [128, 2] f32  ->  outp [rpc, 64] f32

    adj_dtype="fp8": adjt is float8e4 holding adj.T * n (host-scaled into
    [0,1)); the 1/n is folded back in during the ELU pre-scale. H tiles are
    stored fp8 to match the matmul operand dtype.
    """
    import concourse.mybir as mybir

    nc = tc.nc
    dt = mybir.dt
    f32, bf = dt.float32, dt.bfloat16
    AF = mybir.ActivationFunctionType
    ALU = mybir.AluOpType
    fp8 = adj_dtype == "fp8"
    adt = dt.float8e4 if fp8 else bf
    inv_n = 1.0 / n

    rpc = n // n_cores
    kt = n // P
    t_pc = rpc // P
    cw_a = min(1024, n)
    nch_a = n // cw_a
    cw_y = min(512, rpc)
    nch_y = rpc // cw_y
    rest_g = min(8, kt)          # k-tiles per restage DMA
    n_rest = kt // rest_g

    adjt = ins["adjt"]
    zsut = ins["zsut"]
    zat = ins["zat"]
    wbd, wlsu, wlza = ins["wbd"], ins["wlsu"], ins["wlza"]
    blr, bias = ins["blr"], ins["bias"]
    outp = outs["outp"]

    with (
        tc.tile_pool(name="consts", bufs=1) as consts,
        tc.tile_pool(name="hpool", bufs=1) as hpool,
        tc.tile_pool(name="adjp", bufs=4) as adjp,
        tc.tile_pool(name="zp", bufs=3) as zp,
        tc.tile_pool(name="tmp", bufs=2) as tmp,
        tc.tile_pool(name="xp", bufs=1) as xp,
        tc.tile_pool(name="ps", bufs=1, space="PSUM") as ps,
        tc.tile_pool(name="dram", bufs=1, space="DRAM") as dram,
    ):
        # ---- constants to SBUF ----
        wbd_sb = consts.tile([P, L * P], bf, name="wbd_sb")
        nc.sync.dma_start(out=wbd_sb[:], in_=wbd[:])
        wlsu_sb = consts.tile([P, OUT], bf, name="wlsu_sb")
        nc.sync.dma_start(out=wlsu_sb[:], in_=wlsu[:])
        wlza_sb = consts.tile([D, OUT], bf, name="wlza_sb")
        nc.sync.dma_start(out=wlza_sb[:], in_=wlza[:])
        blr_sb = consts.tile([1, OUT], bf, name="blr_sb")
        nc.sync.dma_start(out=blr_sb[:], in_=blr[:])
        bias_sb = consts.tile([P, L], f32, name="bias_sb")
        nc.sync.dma_start(out=bias_sb[:], in_=bias[:])
        zat_sb = consts.tile([D, rpc], bf, name="zat_sb")
        nc.sync.dma_start(out=zat_sb[:], in_=zat[:])
        ones_sb = consts.tile([1, P], bf, name="ones_sb")
        nc.vector.memset(ones_sb[:], 1.0)

        # ---- stage A: E = elu(zsu), H0 = E @ W0bd (node-major) ----
        h0 = hpool.tile([P, n], adt, name="h0", tag="h0")
        for ch in range(nch_a):
            zch = zp.tile([P, cw_a], f32, name="zch", tag="zch")
            nc.sync.dma_start(out=zch[:], in_=zsut[:, ch * cw_a:(ch + 1) * cw_a])
            e_t = tmp.tile([P, cw_a], f32, name="e_t", tag="e_t")
            nc.scalar.activation(e_t[:], zch[:], AF.Exp)
            m_t = tmp.tile([P, cw_a], f32, name="m_t", tag="m_t")
            nc.vector.tensor_scalar_max(m_t[:], zch[:], 0.0)
            x0 = xp.tile([P, cw_a], bf, name="x0", tag="x0", bufs=2)
            nc.vector.scalar_tensor_tensor(
                x0[:], e_t[:], -1.0, m_t[:], op0=ALU.add, op1=ALU.min
            )
            for t in range(cw_a // P):
                g = ch * (cw_a // P) + t
                ph = ps.tile([P, P], f32, name="ph", tag="ph", bufs=2)
                nc.tensor.matmul(
                    ph[:], lhsT=x0[:, t * P:(t + 1) * P], rhs=wbd_sb[:, 0:P],
                    start=True, stop=True,
                )
                nc.vector.tensor_copy(h0[:, g * P:(g + 1) * P], ph[:])

        h_cur = h0
        for layer in range(L):
            # ---- big matmul: Y^T[feat, local nodes] = H^T @ adjT ----
            psy = [
                ps.tile([P, cw_y], f32, name=f"psy{c}", tag=f"psy{c}", bufs=1)
                for c in range(nch_y)
            ]
            for k in range(kt):
                slab = adjp.tile([P, rpc], adt, name="slab", tag="slab")
                nc.sync.dma_start(out=slab[:], in_=adjt[k * P:(k + 1) * P, :])
                for c in range(nch_y):
                    nc.tensor.matmul(
                        psy[c][:],
                        lhsT=h_cur[:, k * P:(k + 1) * P],
                        rhs=slab[:, c * cw_y:(c + 1) * cw_y],
                        start=(k == 0), stop=(k == kt - 1),
                    )

            # ---- X^T = elu(Y^T + b) (feature-major, bf16) ----
            xT = xp.tile([P, rpc], bf, name="xT", tag=f"xT{layer}")
            b_ap = bias_sb[:, layer:layer + 1]
            for c in range(nch_y):
                e_t = tmp.tile([P, cw_y], f32, name="e_t", tag="e_t")
                m_t = tmp.tile([P, cw_y], f32, name="m_t", tag="m_t")
                if fp8:
                    # Y = psy / n ; s = Y + b
                    s_t = tmp.tile([P, cw_y], f32, name="s_t", tag="s_t")
                    nc.vector.tensor_scalar(
                        s_t[:], psy[c][:], inv_n, b_ap, op0=ALU.mult, op1=ALU.add
                    )
                    nc.scalar.activation(e_t[:], s_t[:], AF.Exp)
                    nc.vector.tensor_scalar_max(m_t[:], s_t[:], 0.0)
                else:
                    nc.scalar.activation(e_t[:], psy[c][:], AF.Exp, bias=b_ap)
                    nc.vector.tensor_scalar(
                        m_t[:], psy[c][:], b_ap, 0.0, op0=ALU.add, op1=ALU.max
                    )
                nc.vector.scalar_tensor_tensor(
                    xT[:, c * cw_y:(c + 1) * cw_y], e_t[:], -1.0, m_t[:],
                    op0=ALU.add, op1=ALU.min,
                )

            if layer < L - 1:
                # ---- H1_m = X1 @ W1bd (node-major tiles), AllGather, restage ----
                h1m = xp.tile([P, rpc], adt, name="h1m", tag="h1m")
                for t in range(t_pc):
                    ph = ps.tile([P, P], f32, name="ph", tag="ph", bufs=2)
                    nc.tensor.matmul(
                        ph[:], lhsT=xT[:, t * P:(t + 1) * P],
                        rhs=wbd_sb[:, P:2 * P], start=True, stop=True,
                    )
                    nc.vector.tensor_copy(h1m[:, t * P:(t + 1) * P], ph[:])

                h1 = hpool.tile([P, n], adt, name="h1", tag="h1")
                if with_collective and n_cores > 1:
                    g_in = dram.tile([rpc, P], adt, name="g_in")
                    nc.sync.dma_start(
                        out=g_in.rearrange("(t p) f -> p t f", p=P),
                        in_=h1m.rearrange("p (t f) -> p t f", f=P),
                    )
                    g_out = dram.tile([n, P], adt, name="g_out", addr_space="Shared")
                    nc.gpsimd.collective_compute(
                        "AllGather",
                        mybir.AluOpType.bypass,
                        replica_groups=[list(range(n_cores))],
                        ins=[g_in.opt()],
                        outs=[g_out.opt()],
                    )
                    gview = g_out.rearrange("(k p) f -> p k f", p=P)
                    h1view = h1.rearrange("p (k f) -> p k f", f=P)
                    for g in range(n_rest):
                        nc.sync.dma_start(
                            out=h1view[:, g * rest_g:(g + 1) * rest_g, :],
                            in_=gview[:, g * rest_g:(g + 1) * rest_g, :],
                        )
                else:
                    # cost-model-only path (TimelineSim): emit the same DMA
                    # pattern as the collective path, minus the collective.
                    # Numerically invalid for k-tiles of other cores.
                    g_in = dram.tile([rpc, P], adt, name="g_in")
                    nc.sync.dma_start(
                        out=g_in.rearrange("(t p) f -> p t f", p=P),
                        in_=h1m.rearrange("p (t f) -> p t f", f=P),
                    )
                    g_out = dram.tile([n, P], adt, name="g_out")
                    nc.sync.dma_start(
                        out=g_out[:rpc, :], in_=g_in[:],
                    )
                    gview = g_out.rearrange("(k p) f -> p k f", p=P)
                    h1view = h1.rearrange("p (k f) -> p k f", f=P)
                    for g in range(n_rest):
                        nc.sync.dma_start(
                            out=h1view[:, g * rest_g:(g + 1) * rest_g, :],
                            in_=gview[:, g * rest_g:(g + 1) * rest_g, :],
                        )
                h_cur = h1
            else:
                # ---- final: out = [ys yu] @ Wl[:128] + za @ Wl[128:] + bl ----
                for t in range(t_pc):
                    po = ps.tile([P, OUT], f32, name="po", tag="po", bufs=2)
                    nc.tensor.matmul(
                        po[:], lhsT=xT[:, t * P:(t + 1) * P], rhs=wlsu_sb[:],
                        start=True, stop=False, skip_group_check=True,
                    )
                    nc.tensor.matmul(
                        po[:], lhsT=zat_sb[:, t * P:(t + 1) * P], rhs=wlza_sb[:],
                        start=False, stop=False, skip_group_check=True,
                    )
                    nc.tensor.matmul(
                        po[:], lhsT=ones_sb[:], rhs=blr_sb[:],
                        start=False, stop=True, skip_group_check=True,
                    )
                    ot = zp.tile([P, OUT], f32, name="ot", tag="ot", bufs=2)
                    nc.vector.tensor_copy(ot[:], po[:])
                    nc.sync.dma_start(out=outp[t * P:(t + 1) * P, :], in_=ot[:])


def build_full(n_cores=N_CORES, n=N, adj_dtype="bf16", num_devices=None,
               with_collective=True):
    """Build + compile the full SPMD Bass module (one program, 8 cores)."""
    import concourse.bacc as bacc
    import concourse.mybir as mybir
    import concourse.tile as tile

    dt = mybir.dt
    f32, bf = dt.float32, dt.bfloat16
    adt = dt.float8e4 if adj_dtype == "fp8" else bf
    rpc = n // n_cores
    if num_devices is None:
        num_devices = n_cores

    nc = bacc.Bacc("TRN2", target_bir_lowering=False, debug=False,
                   num_devices=num_devices)
    ins = {
        "adjt": nc.dram_tensor("adjt", [n, rpc], adt, kind="ExternalInput").ap(),
        "zsut": nc.dram_tensor("zsut", [P, n], f32, kind="ExternalInput").ap(),
        "zat": nc.dram_tensor("zat", [D, rpc], bf, kind="ExternalInput").ap(),
        "wbd": nc.dram_tensor("wbd", [P, L * P], bf, kind="ExternalInput").ap(),
        "wlsu": nc.dram_tensor("wlsu", [P, OUT], bf, kind="ExternalInput").ap(),
        "wlza": nc.dram_tensor("wlza", [D, OUT], bf, kind="ExternalInput").ap(),
        "blr": nc.dram_tensor("blr", [1, OUT], bf, kind="ExternalInput").ap(),
        "bias": nc.dram_tensor("bias", [P, L], f32, kind="ExternalInput").ap(),
    }
    outs = {
        "outp": nc.dram_tensor("outp", [rpc, OUT], f32, kind="ExternalOutput").ap(),
    }
    with tile.TileContext(nc) as tc:
        build_kernel_body(tc, ins, outs, n_cores=n_cores, n=n,
                          adj_dtype=adj_dtype, with_collective=with_collective)
    nc.compile()
    return nc


def prep_inputs(z, adj, Ws, bs, Wl, bl, n_cores=N_CORES, n=N, adj_dtype="bf16"):
    """Host-side sharding: build the per-core input maps."""
    rpc = n // n_cores
    z = np.asarray(z, dtype=np.float32)
    adj = np.asarray(adj, dtype=np.float32)
    Ws = np.asarray(Ws, dtype=np.float32)
    bs = np.asarray(bs, dtype=np.float32)
    Wl = np.asarray(Wl, dtype=np.float32)
    bl = np.asarray(bl, dtype=np.float32)

    if adj_dtype == "fp8":
        FP8 = ml_dtypes.float8_e4m3
        adjt = (adj.T * np.float32(n)).astype(FP8)       # [n, n] in [0,1)
    else:
        adjt = np.ascontiguousarray(adj.T).astype(BF16)  # [n, n]
    zsut = np.ascontiguousarray(z[:, :2 * D].T)          # [128, n] f32
    zat = np.ascontiguousarray(z[:, 2 * D:].T).astype(BF16)  # [64, n]

    wbd = np.zeros((P, L * P), dtype=np.float32)
    for l in range(L):
        wbd[:D, l * P:l * P + D] = Ws[l]
        wbd[D:, l * P + D:(l + 1) * P] = Ws[l]
    wbd = wbd.astype(BF16)
    bias = np.stack([np.concatenate([bs[l], bs[l]]) for l in range(L)],
                    axis=1).astype(np.float32)           # [128, L]
    wlsu = np.ascontiguousarray(Wl[:2 * D]).astype(BF16)
    wlza = np.ascontiguousarray(Wl[2 * D:]).astype(BF16)
    blr = bl.reshape(1, OUT).astype(BF16)

    in_maps = []
    for m in range(n_cores):
        in_maps.append({
            "adjt": np.ascontiguousarray(adjt[:, m * rpc:(m + 1) * rpc]),
            "zsut": zsut,
            "zat": np.ascontiguousarray(zat[:, m * rpc:(m + 1) * rpc]),
            "wbd": wbd,
            "wlsu": wlsu,
            "wlza": wlza,
            "blr": blr,
            "bias": bias,
        })
    return in_maps


_NC_CACHE = {}
ADJ_DTYPE = "bf16"


def kernel(z, adj, Ws, bs, Wl, bl):
    """Full-input entry point: shard, run on 8 NeuronCores, gather."""
    from concourse.bass_utils import run_bass_kernel_spmd

    if "nc" not in _NC_CACHE:
        _NC_CACHE["nc"] = build_full(adj_dtype=ADJ_DTYPE)
    nc = _NC_CACHE["nc"]

    in_maps = prep_inputs(z, adj, Ws, bs, Wl, bl, adj_dtype=ADJ_DTYPE)
    res = run_bass_kernel_spmd(nc, in_maps, core_ids=list(range(N_CORES)))
    out = np.concatenate(
        [res.results[m]["outp"] for m in range(N_CORES)], axis=0
    ).astype(np.float32)
    return out
